# revision 5
# baseline (speedup 1.0000x reference)
"""DifferentiableEmbedding kernel for Trainium2 (8 NeuronCores, Bass/Tile).

Semantics (matches the reference nn.Module):
    vec  = embedding[ids]                      [N, D]
    g    = gates[ids]                          [N]
    frac = g*L - floor(g*L)                    (L = 1e9, fp32)
    soft = (frac / L) * tanh(g)
    hard = (arange(D) < g)
    out  = vec * (hard + soft)

Strategy: data-parallel over the 65536 tokens (8192/core); the full table is
replicated to every core's HBM.  The gather uses the SWDGE dma_gather
extended instruction (vectorized Q7 descriptor generation).  dma_gather
indices are int16, so the 128000-row vocab is split into 4 quarters of
<=32768 rows; the host routes each token to its quarter's gather (round-robin
over cores within a quarter keeps per-(core,quarter) counts ~N_q/8).

The table is augmented to 320 f32 columns (row = 256 embedding floats + gate
at col 256 + pad) so one 1280-byte gather element brings the row AND its gate
(dma_gather elem_size must be a multiple of 256 bytes).

Mask math runs on-device: frac via the exact fp32 round-to-nearest-integer
trick (+-2^23), tanh on the scalar (ACT) engine, then per 128-token block
two DVE ops:  mask = (iota < g) + soft  and  out = mask * vec.
"""

import numpy as np

# ---- problem constants (hardcoded per contract) ----
B, S, V, D = 32, 2048, 128000, 256
N = B * S                     # 65536 tokens
NCORES = 8
T = N // NCORES               # 8192 tokens per core
NQ = 4                        # vocab quarters
QROWS = 32768                 # rows per quarter (last quarter: 29696)
C = 2176                      # per-(core,quarter) token capacity (17 blocks)
NBLK = C // 128               # 17
WCOL = C // 16                # 136 idx columns per quarter
ROWW = 320                    # augmented row width (f32 elems); 1280 bytes
TWO23 = 8388608.0             # 2^23
L = 1e9

_cached = {}


def _build_program():
    """Build + compile the SPMD Bass program (same program on all 8 cores)."""
    import concourse.bacc as bacc
    import concourse.tile as tile
    from concourse import mybir

    f32 = mybir.dt.float32
    i16 = mybir.dt.int16
    i32 = mybir.dt.int32

    nc = bacc.Bacc("TRN2", target_bir_lowering=False, debug=False,
                   num_devices=NCORES)

    tbl = nc.dram_tensor("tbl", [V, ROWW], f32, kind="ExternalInput")
    idxs = nc.dram_tensor("idxs", [128, NQ * WCOL], i16, kind="ExternalInput")
    out = nc.dram_tensor("out", [NQ, 128, NBLK * D], f32, kind="ExternalOutput")

    qbounds = [(q * QROWS, min(V, (q + 1) * QROWS)) for q in range(NQ)]

    with tile.TileContext(nc) as tc:
        with (
            tc.tile_pool(name="const", bufs=1) as constp,
            tc.tile_pool(name="rows", bufs=2) as rowsp,
            tc.tile_pool(name="outs", bufs=2) as outsp,
            tc.tile_pool(name="small", bufs=2) as smallp,
            tc.tile_pool(name="mask", bufs=4) as maskp,
        ):
            idx_t = constp.tile([128, NQ * WCOL], i16)
            nc.sync.dma_start(out=idx_t[:], in_=idxs[:])

            iota_i = constp.tile([128, D], i32)
            nc.gpsimd.iota(iota_i[:], pattern=[[1, D]], base=0,
                           channel_multiplier=0)
            iota_f = constp.tile([128, D], f32)
            nc.vector.tensor_copy(out=iota_f[:], in_=iota_i[:])

            for q in range(NQ):
                lo, hi = qbounds[q]
                rows = rowsp.tile([128, NBLK, ROWW], f32)
                # SWDGE descriptor ring fits ~1024 descriptors per gather op
                for c0 in range(0, C, 1024):
                    cn = min(1024, C - c0)
                    nc.gpsimd.dma_gather(
                        out_ap=rows[:, c0 // 128:(c0 + cn) // 128, :],
                        in_ap=tbl[lo:hi, :],
                        idxs_ap=idx_t[:, (q * C + c0) // 16:(q * C + c0 + cn) // 16],
                        num_idxs=cn,
                        num_idxs_reg=cn,
                        elem_size=ROWW,
                        queue_num=0,
                    )

                g = rows[:, :, 256]                      # [128, NBLK] stride 320
                # soft = (frac(g*L) / L) * tanh(g), exact fp32 reproduction
                t = smallp.tile([128, NBLK], f32, tag="t")
                nc.vector.tensor_scalar_mul(t[:], g, float(L))
                tcl = smallp.tile([128, NBLK], f32, tag="tcl")
                nc.vector.tensor_scalar_min(tcl[:], t[:], TWO23)
                a = smallp.tile([128, NBLK], f32, tag="a")
                nc.vector.tensor_scalar_add(a[:], tcl[:], TWO23)
                b = smallp.tile([128, NBLK], f32, tag="b")
                nc.vector.tensor_scalar_sub(b[:], a[:], TWO23)
                cgt = smallp.tile([128, NBLK], f32, tag="cgt")
                nc.vector.tensor_tensor(out=cgt[:], in0=b[:], in1=tcl[:],
                                        op=mybir.AluOpType.is_gt)
                fl = smallp.tile([128, NBLK], f32, tag="fl")
                nc.vector.tensor_tensor(out=fl[:], in0=b[:], in1=cgt[:],
                                        op=mybir.AluOpType.subtract)
                fr = smallp.tile([128, NBLK], f32, tag="fr")
                nc.vector.tensor_tensor(out=fr[:], in0=tcl[:], in1=fl[:],
                                        op=mybir.AluOpType.subtract)
                th = smallp.tile([128, NBLK], f32, tag="th")
                nc.scalar.activation(th[:], g,
                                     mybir.ActivationFunctionType.Tanh)
                soft = smallp.tile([128, NBLK], f32, tag="soft")
                nc.vector.scalar_tensor_tensor(
                    out=soft[:], in0=fr[:], scalar=1e-9, in1=th[:],
                    op0=mybir.AluOpType.mult, op1=mybir.AluOpType.mult)

                ot = outsp.tile([128, NBLK, D], f32)
                for blk in range(NBLK):
                    mask = maskp.tile([128, D], f32)
                    nc.vector.tensor_scalar(
                        out=mask[:], in0=iota_f[:],
                        scalar1=rows[:, blk, 256:257],
                        scalar2=soft[:, blk:blk + 1],
                        op0=mybir.AluOpType.is_lt,
                        op1=mybir.AluOpType.add)
                    nc.vector.tensor_tensor(
                        out=ot[:, blk, :], in0=mask[:],
                        in1=rows[:, blk, 0:D],
                        op=mybir.AluOpType.mult)

                nc.sync.dma_start(out=out[q],
                                  in_=ot[:].rearrange("p a b -> p (a b)"))

    nc.compile()
    return nc


def _host_shard(input_ids, embedding, gates):
    """Build per-core device inputs + reassembly metadata."""
    ids = np.ascontiguousarray(input_ids).reshape(-1).astype(np.int64)
    assert ids.shape[0] == N

    aug = np.zeros((V, ROWW), dtype=np.float32)
    aug[:, :D] = np.asarray(embedding, dtype=np.float32)
    aug[:, D] = np.asarray(gates, dtype=np.float32)

    idx_arrs = [np.zeros((128, NQ * WCOL), dtype=np.int16) for _ in range(NCORES)]
    # token positions (into flat ids) per (core, quarter), in gather order
    tok_pos = [[None] * NQ for _ in range(NCORES)]

    for q in range(NQ):
        lo = q * QROWS
        hi = min(V, lo + QROWS)
        pos_q = np.flatnonzero((ids >= lo) & (ids < hi))
        for c in range(NCORES):
            pos_cq = pos_q[c::NCORES]
            n = pos_cq.shape[0]
            if n > C:
                raise ValueError(
                    f"quarter {q} core {c}: {n} tokens exceeds capacity {C}")
            tok_pos[c][q] = pos_cq
            idx16 = np.zeros(C, dtype=np.int16)
            idx16[:n] = (ids[pos_cq] - lo).astype(np.int16)
            # wrap: logical j -> partition j%16, column j//16; replicate x8
            w = idx16.reshape(WCOL, 16).T                      # [16, WCOL]
            idx_arrs[c][:, q * WCOL:(q + 1) * WCOL] = np.tile(w, (8, 1))

    return aug, idx_arrs, tok_pos


def _unshard(results, tok_pos):
    out_full = np.empty((N, D), dtype=np.float32)
    for c in range(NCORES):
        dev = results[c]["out"].reshape(NQ, 128, NBLK, D)
        for q in range(NQ):
            pos = tok_pos[c][q]
            n = pos.shape[0]
            if n == 0:
                continue
            # token j of this (core, quarter) group lives at
            # partition j%128, block j//128
            rows = dev[q].transpose(1, 0, 2).reshape(C, D)
            out_full[pos] = rows[:n]
    return out_full.reshape(B, S, D)


def kernel(input_ids, embedding, gates):
    from concourse.bass_utils import run_bass_kernel_spmd

    if "nc" not in _cached:
        _cached["nc"] = _build_program()
    nc = _cached["nc"]

    aug, idx_arrs, tok_pos = _host_shard(input_ids, embedding, gates)
    in_maps = [{"tbl": aug, "idxs": idx_arrs[c]} for c in range(NCORES)]
    res = run_bass_kernel_spmd(nc, in_maps, list(range(NCORES)))
    return _unshard(res.results, tok_pos)
